# revision 13
# baseline (speedup 1.0000x reference)
"""EntropyGraph Trainium2 kernel.

Computes, per batch b (one NeuronCore per batch):
  qt = heads(queries @ Wq_w.T + Wq_b), kt = heads(keys @ Wk_w.T + Wk_b)
  out[b,h,i,j] = -0.5 * sum_m log(1 - corr_m(i,j)^2 + eps)
where corr_m is the lag-m cross-correlation between query series i and key
series j within each head.

Strategy: per head and lag m, the PE computes a Gram G_m = X_m^T Y_m with
one-sided mean augmentation (aug rows make the centering exact) and the
k-side 1/sqrt(ssy) normalization folded into Y. Then
rho_m^2 = G_m^2 / ssx_m, and
  out = -0.5*ln[(C - rho1^2)(C - rho2^2)]
      = -0.5*ln[(C1' - (cG1)^2)(C2' - (cG2)^2)] + D
with Cm' = C*c^2*ssx_m (per-partition) and D = 0.5*[ln(c^2 ssx_1) +
ln(c^2 ssx_2)] (per-partition), c a fixed constant. This keeps the PSUM
evacuations as plain constant-scale ACT Squares over [128,2048] paired
tiles (two i-chunks per op, amortizing ACT's fixed overhead), and the
rest of the elementwise chain as fast f16 2x-mode DVE tensor_scalar /
tensor_tensor ops. Everything downstream of PSUM is f16.
"""

import sys

import numpy as np

sys.path.insert(0, "/opt/trn_rl_repo")

import concourse.bacc as bacc
import concourse.tile as tile
from concourse import mybir
from concourse.bass_utils import run_bass_kernel_spmd

F32 = mybir.dt.float32
F16 = mybir.dt.float16
OP = mybir.AluOpType
AF = mybir.ActivationFunctionType

B, N, DF = 8, 1024, 128
H, DK = 8, 64
EPS = 1e-6
C = 1.0 + EPS
CSCALE = 0.25          # constant Gram prescale c
NCHUNK = 4             # o-chunks of 128 in the 512-wide projection


def _build_nc():
    nc = bacc.Bacc("TRN2", target_bir_lowering=False)

    qT = nc.dram_tensor("qT", [DF, N], F16, kind="ExternalInput")
    kT = nc.dram_tensor("kT", [DF, N], F16, kind="ExternalInput")
    wqT = nc.dram_tensor("wqT", [DF, 512], F16, kind="ExternalInput")
    wkT = nc.dram_tensor("wkT", [DF, 512], F16, kind="ExternalInput")
    bq = nc.dram_tensor("bq", [128, 4], F32, kind="ExternalInput")
    bk = nc.dram_tensor("bk", [128, 4], F32, kind="ExternalInput")
    xmask = nc.dram_tensor("xmask", [128, 64], F16, kind="ExternalInput")
    ymask = nc.dram_tensor("ymask", [128, 64], F16, kind="ExternalInput")
    invn = nc.dram_tensor("invn", [16, 1], F32, kind="ExternalInput")
    ident = nc.dram_tensor("ident", [16, 16], F32, kind="ExternalInput")
    out = nc.dram_tensor("out", [H, N, N], F16, kind="ExternalOutput")
    # DRAM bounce buffer for beta: SBUF sources cannot use partition-step-0
    # (broadcast) APs, DRAM sources can.
    betad = nc.dram_tensor("betad", [16, N], F16, kind="Internal")

    with tile.TileContext(nc) as tc:
        with tc.tile_pool(name="const", bufs=1) as const, \
             tc.tile_pool(name="proj", bufs=1) as projp, \
             tc.tile_pool(name="stats", bufs=1) as statp, \
             tc.tile_pool(name="scratch", bufs=2) as scratch:

            # ---- Stage A: load inputs -------------------------------------
            qT_s = const.tile([DF, N], F16)
            kT_s = const.tile([DF, N], F16)
            wqT_s = const.tile([DF, 512], F16)
            wkT_s = const.tile([DF, 512], F16)
            bq_s = const.tile([128, 4], F32)
            bk_s = const.tile([128, 4], F32)
            xm_s = const.tile([128, 64], F16)
            ym_s = const.tile([128, 64], F16)
            invn_s = const.tile([16, 1], F32)
            id_s = const.tile([16, 16], F32)
            for dst, src in ((qT_s, qT), (kT_s, kT), (wqT_s, wqT),
                             (wkT_s, wkT), (bq_s, bq), (bk_s, bk),
                             (xm_s, xmask), (ym_s, ymask), (invn_s, invn),
                             (id_s, ident)):
                nc.sync.dma_start(out=dst, in_=src[:, :])

            # ---- Stage B: projections (transposed layout) -----------------
            # projT[o, n] = W[o, :] @ inT[:, n] + b[o]; f16 matmul, ACT evac
            # adds the per-partition bias and rounds to f16.
            qproj = []
            kproj = []
            with tc.tile_pool(name="pps", bufs=2, space="PSUM") as pps:
                for (src_s, w_s, b_s, dst_list) in (
                        (kT_s, wkT_s, bk_s, kproj),
                        (qT_s, wqT_s, bq_s, qproj)):
                    for c in range(NCHUNK):
                        psb = pps.tile([128, N], F32)
                        for jh in range(2):
                            nc.tensor.matmul(
                                psb[:, jh * 512:(jh + 1) * 512],
                                lhsT=w_s[:, c * 128:(c + 1) * 128],
                                rhs=src_s[:, jh * 512:(jh + 1) * 512],
                                start=True, stop=True)
                        pt = projp.tile([128, N], F16, tag=f"proj{len(dst_list)}_{c}_{w_s is wkT_s}")
                        nc.scalar.activation(out=pt, in_=psb, func=AF.Identity,
                                             bias=b_s[:, c:c + 1], scale=1.0)
                        dst_list.append(pt)

            # ---- Stage C: raw moments via mask matmuls --------------------
            # s1[r, n] = sum_d proj[d, n] over the (head, m) range; r = 2h+m-1
            # s2 likewise over squared projections.
            sq_list = {}
            with tc.tile_pool(name="sqp", bufs=2) as sqp:
                for name, plist in (("k", kproj), ("q", qproj)):
                    for c in range(NCHUNK):
                        sq = sqp.tile([128, N], F16, tag=f"sq{name}{c}")
                        nc.vector.tensor_tensor(out=sq, in0=plist[c],
                                                in1=plist[c], op=OP.mult)
                        sq_list[(name, c)] = sq

                stats_sb = {}
                with tc.tile_pool(name="sps", bufs=1, space="PSUM") as sps:
                    for name, plist, mask in (("k", kproj, ym_s),
                                              ("q", qproj, xm_s)):
                        ps1 = sps.tile([16, N], F32, tag=f"ps1{name}")
                        ps2 = sps.tile([16, N], F32, tag=f"ps2{name}")
                        for c in range(NCHUNK):
                            for jh in range(2):
                                sl = slice(jh * 512, (jh + 1) * 512)
                                nc.tensor.matmul(
                                    ps1[:, sl],
                                    lhsT=mask[:, 16 * c:16 * c + 16],
                                    rhs=plist[c][:, sl],
                                    start=(c == 0), stop=(c == NCHUNK - 1))
                                nc.tensor.matmul(
                                    ps2[:, sl],
                                    lhsT=mask[:, 16 * c:16 * c + 16],
                                    rhs=sq_list[(name, c)][:, sl],
                                    start=(c == 0), stop=(c == NCHUNK - 1))
                        s1 = statp.tile([16, N], F32, tag=f"s1{name}")
                        s2 = statp.tile([16, N], F32, tag=f"s2{name}")
                        nc.scalar.copy(s1, ps1)
                        nc.scalar.copy(s2, ps2)
                        stats_sb[name] = (s1, s2)

            # ---- Stage D: stats math --------------------------------------
            s1q, s2q = stats_sb["q"]
            s1k, s2k = stats_sb["k"]
            invn_ap = invn_s[:, 0:1]

            # ns1y = -s1y (f16, aug rows) -- early: yraw DMAs depend on it
            ns1y = statp.tile([16, N], F16)
            nc.vector.tensor_scalar(out=ns1y, in0=s1k, scalar1=-1.0,
                                    scalar2=None, op0=OP.mult)
            # k-side: beta = 1/sqrt(ssy) via Abs_reciprocal_sqrt (measured
            # 4e-5 max rel err) -- avoids the 6.5us DVE reciprocal on the
            # critical path to the first Gram matmul.
            tk = scratch.tile([16, N], F32, tag="tk")
            nc.vector.tensor_mul(tk, s1k, s1k)
            nssy = scratch.tile([16, N], F32, tag="nssy")
            nc.vector.scalar_tensor_tensor(out=nssy, in0=tk, scalar=invn_ap,
                                           in1=s2k, op0=OP.mult, op1=OP.subtract)
            beta16 = statp.tile([16, N], F16)
            nc.scalar.activation(out=beta16, in_=nssy,
                                 func=AF.Abs_reciprocal_sqrt,
                                 bias=0.0, scale=-1.0)
            nc.sync.dma_start(out=betad[:, :], in_=beta16)

            # mx = s1x / n_eff  (f16: aug rows)
            mx = statp.tile([16, N], F16)
            nc.vector.tensor_scalar(out=mx, in0=s1q, scalar1=invn_ap,
                                    scalar2=None, op0=OP.mult)
            # -ssx = s1x^2/n - s2x
            tq = scratch.tile([16, N], F32, tag="tq")
            nc.vector.tensor_mul(tq, s1q, s1q)
            nssx = scratch.tile([16, N], F32, tag="nssx")
            nc.vector.scalar_tensor_tensor(out=nssx, in0=tq, scalar=invn_ap,
                                           in1=s2q, op0=OP.mult, op1=OP.subtract)
            # Ca = C*c^2*ssx  (additive constant for h = Ca - (c*G)^2)
            caf = scratch.tile([16, N], F32, tag="caf")
            nc.vector.tensor_scalar(out=caf, in0=nssx,
                                    scalar1=-(C * CSCALE * CSCALE),
                                    scalar2=None, op0=OP.mult)
            # LN = ln(c^2*ssx)  (log correction, per (r, i))
            lnf = scratch.tile([16, N], F32, tag="lnf")
            nc.scalar.activation(out=lnf, in_=nssx, func=AF.Ln,
                                 bias=0.0, scale=-(CSCALE * CSCALE))

            # transpose Ca and LN to [128, 8*16]: col ic*16 + (2h + m - 1)
            caT = statp.tile([128, 128], F32)
            lnT = statp.tile([128, 128], F32)
            with tc.tile_pool(name="tps", bufs=1, space="PSUM") as tps:
                pst = tps.tile([128, 128], F32, tag="pstA")
                for ic in range(8):
                    nc.tensor.transpose(pst[:, ic * 16:(ic + 1) * 16],
                                        in_=caf[:, ic * 128:(ic + 1) * 128],
                                        identity=id_s[0:16, 0:16])
                nc.scalar.copy(caT, pst)
                pst2 = tps.tile([128, 128], F32, tag="pstB")
                for ic in range(8):
                    nc.tensor.transpose(pst2[:, ic * 16:(ic + 1) * 16],
                                        in_=lnf[:, ic * 128:(ic + 1) * 128],
                                        identity=id_s[0:16, 0:16])
                nc.scalar.copy(lnT, pst2)
            # D[i, ic*8+h] = 0.5*(LN[2h] + LN[2h+1])  (free-dim stride-2 APs)
            dT = statp.tile([128, 64], F32)
            nc.vector.tensor_tensor(out=dT, in0=lnT[:, 0:128:2],
                                    in1=lnT[:, 1:128:2], op=OP.add)
            nc.vector.tensor_scalar(out=dT, in0=dT, scalar1=0.5,
                                    scalar2=None, op0=OP.mult)

            # m1 augmentation: overwrite q_projT row rb+63 (unused d=63) with mx1
            for h in range(H):
                ch, rb = h // 2, (h % 2) * 64
                nc.sync.dma_start(out=qproj[ch][rb + 63:rb + 64, :],
                                    in_=mx[2 * h:2 * h + 1, :])

            # ---- Stage E: per-head Grams + elementwise --------------------
            prev = None
            with tc.tile_pool(name="head", bufs=2) as headp, \
                 tc.tile_pool(name="nsq", bufs=2) as nsq, \
                 tc.tile_pool(name="gps", bufs=1, space="PSUM") as gps:
                for h in range(H):
                    ch, rb = h // 2, (h % 2) * 64
                    yo1, yo2 = rb, 64 - rb
                    r1, r2 = 2 * h, 2 * h + 1

                    # Y raw: m1 block rows yo1..yo1+63 (k d=1..63 + aug),
                    #        m2 block rows yo2..yo2+62 (k d=2..63 + aug)
                    yraw = headp.tile([128, N], F16, tag="yraw")
                    nc.sync.dma_start(out=yraw[yo1:yo1 + 63, :],
                                        in_=kproj[ch][rb + 1:rb + 64, :])
                    nc.sync.dma_start(out=yraw[yo1 + 63:yo1 + 64, :],
                                        in_=ns1y[r1:r1 + 1, :])
                    nc.sync.dma_start(out=yraw[yo2:yo2 + 62, :],
                                        in_=kproj[ch][rb + 2:rb + 64, :])
                    nc.sync.dma_start(out=yraw[yo2 + 62:yo2 + 63, :],
                                        in_=ns1y[r2:r2 + 1, :])
                    hole = yo2 + 63  # the single uncovered row
                    nc.sync.dma_start(out=yraw[hole:hole + 1, :],
                                        in_=ns1y[r1:r1 + 1, :])

                    bb = headp.tile([128, N], F16, tag="bb")
                    nc.gpsimd.dma_start(
                        out=bb[yo1:yo1 + 64, :],
                        in_=betad[r1:r1 + 1, :].to_broadcast((64, N)))
                    nc.gpsimd.dma_start(
                        out=bb[yo2:yo2 + 64, :],
                        in_=betad[r2:r2 + 1, :].to_broadcast((64, N)))

                    yt = headp.tile([128, N], F16, tag="yt")
                    nc.vector.tensor_mul(yt, yraw, bb)

                    # X2: m2 lhsT block at rows yo2..yo2+62 (q d=0..61 + mx2)
                    x2 = headp.tile([128, N], F16, tag="x2")
                    nc.sync.dma_start(out=x2[yo2:yo2 + 62, :],
                                        in_=qproj[ch][rb:rb + 62, :])
                    nc.sync.dma_start(out=x2[yo2 + 62:yo2 + 63, :],
                                        in_=mx[r2:r2 + 1, :])

                    for pc in range(4):
                        ic0, ic1 = 2 * pc, 2 * pc + 1
                        psA = gps.tile([128, 2 * N], F32, tag="psA")
                        psB = gps.tile([128, 2 * N], F32, tag="psB")
                        for k, ic in ((0, ic0), (1, ic1)):
                            isl = slice(ic * 128, (ic + 1) * 128)
                            for jh in range(2):
                                jsl = slice(jh * 512, (jh + 1) * 512)
                                osl = slice(k * N + jh * 512,
                                            k * N + (jh + 1) * 512)
                                nc.tensor.matmul(psA[:, osl],
                                                 lhsT=qproj[ch][rb:rb + 64, isl],
                                                 rhs=yt[yo1:yo1 + 64, jsl],
                                                 start=True, stop=True)
                        for k, ic in ((0, ic0), (1, ic1)):
                            isl = slice(ic * 128, (ic + 1) * 128)
                            for jh in range(2):
                                jsl = slice(jh * 512, (jh + 1) * 512)
                                osl = slice(k * N + jh * 512,
                                            k * N + (jh + 1) * 512)
                                nc.tensor.matmul(psB[:, osl],
                                                 lhsT=x2[yo2:yo2 + 63, isl],
                                                 rhs=yt[yo2:yo2 + 63, jsl],
                                                 start=True, stop=True)

                        # sq_m = (c*G_m)^2, [128, 2048].  Every 3rd pair the
                        # G2 evacuation goes via DVE to offload the ACT
                        # engine (the bottleneck).
                        sg1 = nsq.tile([128, 2 * N], F16, tag="sg1")
                        nc.scalar.activation(out=sg1, in_=psA, func=AF.Square,
                                             bias=0.0, scale=CSCALE)
                        sg2 = nsq.tile([128, 2 * N], F16, tag="sg2")
                        if (h * 4 + pc) % 8 in (2, 5, 7):
                            e2 = nsq.tile([128, 2 * N], F16, tag="e2")
                            nc.vector.tensor_scalar(out=e2, in0=psB,
                                                    scalar1=CSCALE,
                                                    scalar2=None, op0=OP.mult)
                            nc.vector.tensor_tensor(out=sg2, in0=e2, in1=e2,
                                                    op=OP.mult)
                        else:
                            nc.scalar.activation(out=sg2, in_=psB,
                                                 func=AF.Square,
                                                 bias=0.0, scale=CSCALE)

                        # retire previous 2-pair group (one Ln over [128,4096])
                        # right after the new group's first sq emissions so
                        # the ACT queue never stalls on DVE
                        if prev is not None and pc % 2 == 0:
                            _retire(nc, nsq, out, dT, prev)
                            prev = None
                        if pc % 2 == 0:
                            ucur = nsq.tile([128, 4 * N], F16, tag="u4")
                            uinfo = []

                        # h_m = Ca_m - sq_m ; u = h1*h2
                        h1 = nsq.tile([128, 2 * N], F16, tag="h1")
                        h2 = nsq.tile([128, 2 * N], F16, tag="h2")
                        for k, ic in ((0, ic0), (1, ic1)):
                            ksl = slice(k * N, (k + 1) * N)
                            nc.vector.tensor_scalar(
                                out=h1[:, ksl], in0=sg1[:, ksl], scalar1=-1.0,
                                scalar2=caT[:, ic * 16 + r1:ic * 16 + r1 + 1],
                                op0=OP.mult, op1=OP.add)
                            nc.vector.tensor_scalar(
                                out=h2[:, ksl], in0=sg2[:, ksl], scalar1=-1.0,
                                scalar2=caT[:, ic * 16 + r2:ic * 16 + r2 + 1],
                                op0=OP.mult, op1=OP.add)
                        usl = slice((pc % 2) * 2 * N, (pc % 2 + 1) * 2 * N)
                        nc.vector.tensor_tensor(out=ucur[:, usl], in0=h1,
                                                in1=h2, op=OP.mult)
                        uinfo.extend([ic0, ic1])
                        if pc % 2 == 1:
                            prev = (ucur, h, uinfo)
                    if h == H - 1 and prev is not None:
                        _retire(nc, nsq, out, dT, prev)
                        prev = None
    nc.compile()
    return nc


def _retire(nc, nsq, out, dT, prev):
    """Ln + final scale/correction + store for a finished 2-pair group."""
    u4, h, ics = prev
    lt = nsq.tile([128, 4 * N], F16, tag="lt")
    nc.scalar.activation(out=lt, in_=u4, func=AF.Ln, bias=0.0, scale=1.0)
    # o = -0.5*ln(u) + D  (per-partition D differs per i-chunk)
    for k, ic in enumerate(ics):
        ksl = slice(k * N, (k + 1) * N)
        o = nsq.tile([128, N], F16, tag=f"o{k}")
        nc.vector.tensor_scalar(
            out=o, in0=lt[:, ksl], scalar1=-0.5,
            scalar2=dT[:, ic * 8 + h:ic * 8 + h + 1],
            op0=OP.mult, op1=OP.add)
        isl = slice(ic * 128, (ic + 1) * 128)
        nc.sync.dma_start(out=out[h, isl, :], in_=o)


_NC = None


def _get_nc():
    global _NC
    if _NC is None:
        _NC = _build_nc()
    return _NC


def _host_inputs(queries, keys, Wq_w, Wq_b, Wk_w, Wk_b):
    qT = np.ascontiguousarray(queries.transpose(0, 2, 1), dtype=np.float16)
    kT = np.ascontiguousarray(keys.transpose(0, 2, 1), dtype=np.float16)
    wqT = np.ascontiguousarray(Wq_w.T, dtype=np.float16)
    wkT = np.ascontiguousarray(Wk_w.T, dtype=np.float16)
    bq = np.ascontiguousarray(Wq_b.reshape(4, 128).T, dtype=np.float32)
    bk = np.ascontiguousarray(Wk_b.reshape(4, 128).T, dtype=np.float32)

    xmask = np.zeros((128, 64), dtype=np.float16)
    ymask = np.zeros((128, 64), dtype=np.float16)
    for c in range(4):
        for hp in range(2):
            for m in (1, 2):
                j = 4 * c + 2 * hp + (m - 1)      # output partition row r
                col = 16 * c + j                   # column within this chunk's mask
                rows = np.arange(hp * 64, hp * 64 + 64 - m)
                xmask[rows, col] = 1.0
                yrows = np.arange(hp * 64 + m, hp * 64 + 64)
                ymask[yrows, col] = 1.0

    invn = np.array([[1.0 / (64 - ((r % 2) + 1))] for r in range(16)],
                    dtype=np.float32)
    ident = np.eye(16, dtype=np.float32)

    shared = dict(wqT=wqT, wkT=wkT, bq=bq, bk=bk, xmask=xmask, ymask=ymask,
                  invn=invn, ident=ident)
    in_maps = []
    for b in range(B):
        m = dict(shared)
        m["qT"] = np.ascontiguousarray(qT[b])
        m["kT"] = np.ascontiguousarray(kT[b])
        in_maps.append(m)
    return in_maps


def kernel(queries, keys, Wq_w, Wq_b, Wk_w, Wk_b):
    nc = _get_nc()
    in_maps = _host_inputs(np.asarray(queries), np.asarray(keys),
                           np.asarray(Wq_w), np.asarray(Wq_b),
                           np.asarray(Wk_w), np.asarray(Wk_b))
    res = run_bass_kernel_spmd(nc, in_maps, core_ids=list(range(B)))
    out = np.stack([res.results[b]["out"].astype(np.float32) for b in range(B)],
                   axis=0)
    return out


# revision 18
# speedup vs baseline: 1.0169x; 1.0169x over previous
"""EntropyGraph Trainium2 kernel.

Computes, per batch b (one NeuronCore per batch):
  qt = heads(queries @ Wq_w.T + Wq_b), kt = heads(keys @ Wk_w.T + Wk_b)
  out[b,h,i,j] = -0.5 * sum_m log(1 - corr_m(i,j)^2 + eps)
where corr_m is the lag-m cross-correlation between query series i and key
series j within each head.

Strategy: per head and lag m, the PE computes a Gram G_m = X_m^T Y_m with
one-sided mean augmentation (aug rows make the centering exact) and the
k-side 1/sqrt(ssy) normalization folded into Y. Then
rho_m^2 = G_m^2 / ssx_m, and
  out = -0.5*ln[(C - rho1^2)(C - rho2^2)]
      = -0.5*ln[(C1' - (cG1)^2)(C2' - (cG2)^2)] + D
with Cm' = C*c^2*ssx_m (per-partition) and D = 0.5*[ln(c^2 ssx_1) +
ln(c^2 ssx_2)] (per-partition), c a fixed constant. This keeps the PSUM
evacuations as plain constant-scale ACT Squares over [128,2048] paired
tiles (two i-chunks per op, amortizing ACT's fixed overhead), and the
rest of the elementwise chain as fast f16 2x-mode DVE tensor_scalar /
tensor_tensor ops. Everything downstream of PSUM is f16.
"""

import sys

import numpy as np

sys.path.insert(0, "/opt/trn_rl_repo")

import concourse.bacc as bacc
import concourse.tile as tile
from concourse import mybir
from concourse.bass_utils import run_bass_kernel_spmd

F32 = mybir.dt.float32
F16 = mybir.dt.float16
OP = mybir.AluOpType
AF = mybir.ActivationFunctionType

B, N, DF = 8, 1024, 128
H, DK = 8, 64
EPS = 1e-6
C = 1.0 + EPS
CSCALE = 0.25          # constant Gram prescale c
NCHUNK = 4             # o-chunks of 128 in the 512-wide projection


def _build_nc():
    nc = bacc.Bacc("TRN2", target_bir_lowering=False)

    qT = nc.dram_tensor("qT", [DF, N], F16, kind="ExternalInput")
    kT = nc.dram_tensor("kT", [DF, N], F16, kind="ExternalInput")
    wqT = nc.dram_tensor("wqT", [DF, 512], F16, kind="ExternalInput")
    wkT = nc.dram_tensor("wkT", [DF, 512], F16, kind="ExternalInput")
    bq = nc.dram_tensor("bq", [128, 4], F32, kind="ExternalInput")
    bk = nc.dram_tensor("bk", [128, 4], F32, kind="ExternalInput")
    xmask = nc.dram_tensor("xmask", [128, 64], F16, kind="ExternalInput")
    ymask = nc.dram_tensor("ymask", [128, 64], F16, kind="ExternalInput")
    invn = nc.dram_tensor("invn", [16, 1], F32, kind="ExternalInput")
    ident = nc.dram_tensor("ident", [16, 16], F32, kind="ExternalInput")
    out = nc.dram_tensor("out", [H, N, N], F16, kind="ExternalOutput")
    # DRAM bounce buffer for beta: SBUF sources cannot use partition-step-0
    # (broadcast) APs, DRAM sources can.
    betad = nc.dram_tensor("betad", [16, N], F16, kind="Internal")

    with tile.TileContext(nc) as tc:
        with tc.tile_pool(name="const", bufs=1) as const, \
             tc.tile_pool(name="proj", bufs=1) as projp, \
             tc.tile_pool(name="stats", bufs=1) as statp, \
             tc.tile_pool(name="scratch", bufs=2) as scratch:

            # ---- Stage A: load inputs -------------------------------------
            qT_s = const.tile([DF, N], F16)
            kT_s = const.tile([DF, N], F16)
            wqT_s = const.tile([DF, 512], F16)
            wkT_s = const.tile([DF, 512], F16)
            bq_s = const.tile([128, 4], F32)
            bk_s = const.tile([128, 4], F32)
            xm_s = const.tile([128, 64], F16)
            ym_s = const.tile([128, 64], F16)
            invn_s = const.tile([16, 1], F32)
            id_s = const.tile([16, 16], F32)
            for dst, src in ((qT_s, qT), (kT_s, kT), (wqT_s, wqT),
                             (wkT_s, wkT), (bq_s, bq), (bk_s, bk),
                             (xm_s, xmask), (ym_s, ymask), (invn_s, invn),
                             (id_s, ident)):
                nc.sync.dma_start(out=dst, in_=src[:, :])

            # ---- Stages B+C per side: projections, then raw moments -------
            # K side fully first so the beta chain starts while the q-side
            # projections still run on the PE.
            # projT[o, n] = W[o, :] @ inT[:, n] + b[o]; f16 matmul, ACT evac
            # adds the per-partition bias and rounds to f16.
            # s1[r, n] = sum_d proj[d, n] over the (head, m) range; r = 2h+m-1
            # s2 likewise over squared projections.
            qproj = []
            kproj = []
            stats_sb = {}
            with tc.tile_pool(name="pps", bufs=2, space="PSUM") as pps, \
                 tc.tile_pool(name="sqp", bufs=2) as sqp, \
                 tc.tile_pool(name="sps", bufs=1, space="PSUM") as sps:
                for (name, src_s, w_s, b_s, mask, dst_list) in (
                        ("k", kT_s, wkT_s, bk_s, ym_s, kproj),
                        ("q", qT_s, wqT_s, bq_s, xm_s, qproj)):
                    for c in range(NCHUNK):
                        psb = pps.tile([128, N], F32)
                        for jh in range(2):
                            nc.tensor.matmul(
                                psb[:, jh * 512:(jh + 1) * 512],
                                lhsT=w_s[:, c * 128:(c + 1) * 128],
                                rhs=src_s[:, jh * 512:(jh + 1) * 512],
                                start=True, stop=True)
                        pt = projp.tile([128, N], F16, tag=f"proj_{name}_{c}")
                        nc.scalar.activation(out=pt, in_=psb, func=AF.Identity,
                                             bias=b_s[:, c:c + 1], scale=1.0)
                        dst_list.append(pt)
                    sq_side = []
                    for c in range(NCHUNK):
                        sq = sqp.tile([128, N], F16, tag=f"sq{name}{c}")
                        nc.vector.tensor_tensor(out=sq, in0=dst_list[c],
                                                in1=dst_list[c], op=OP.mult)
                        sq_side.append(sq)
                    ps1 = sps.tile([16, N], F32, tag="ps1")
                    ps2 = sps.tile([16, N], F32, tag="ps2")
                    for c in range(NCHUNK):
                        for jh in range(2):
                            sl = slice(jh * 512, (jh + 1) * 512)
                            nc.tensor.matmul(
                                ps1[:, sl],
                                lhsT=mask[:, 16 * c:16 * c + 16],
                                rhs=dst_list[c][:, sl],
                                start=(c == 0), stop=(c == NCHUNK - 1))
                            nc.tensor.matmul(
                                ps2[:, sl],
                                lhsT=mask[:, 16 * c:16 * c + 16],
                                rhs=sq_side[c][:, sl],
                                start=(c == 0), stop=(c == NCHUNK - 1))
                    s1 = statp.tile([16, N], F32, tag=f"s1{name}")
                    s2 = statp.tile([16, N], F32, tag=f"s2{name}")
                    nc.scalar.copy(s1, ps1)
                    nc.scalar.copy(s2, ps2)
                    stats_sb[name] = (s1, s2)

            # ---- Stage D: stats math --------------------------------------
            s1q, s2q = stats_sb["q"]
            s1k, s2k = stats_sb["k"]
            invn_ap = invn_s[:, 0:1]

            # ns1y = -s1y (f16, aug rows) -- early: yraw DMAs depend on it
            ns1y = statp.tile([16, N], F16)
            nc.vector.tensor_scalar(out=ns1y, in0=s1k, scalar1=-1.0,
                                    scalar2=None, op0=OP.mult)
            # k-side: beta = 1/sqrt(ssy) via Abs_reciprocal_sqrt (measured
            # 4e-5 max rel err) -- avoids the 6.5us DVE reciprocal on the
            # critical path to the first Gram matmul.
            tk = scratch.tile([16, N], F32, tag="tk")
            nc.vector.tensor_mul(tk, s1k, s1k)
            nssy = scratch.tile([16, N], F32, tag="nssy")
            nc.vector.scalar_tensor_tensor(out=nssy, in0=tk, scalar=invn_ap,
                                           in1=s2k, op0=OP.mult, op1=OP.subtract)
            beta16 = statp.tile([16, N], F16)
            nc.scalar.activation(out=beta16, in_=nssy,
                                 func=AF.Abs_reciprocal_sqrt,
                                 bias=0.0, scale=-1.0)
            nc.sync.dma_start(out=betad[:, :], in_=beta16)

            # mx = s1x / n_eff  (f16: aug rows)
            mx = statp.tile([16, N], F16)
            nc.vector.tensor_scalar(out=mx, in0=s1q, scalar1=invn_ap,
                                    scalar2=None, op0=OP.mult)
            # -ssx = s1x^2/n - s2x
            tq = scratch.tile([16, N], F32, tag="tq")
            nc.vector.tensor_mul(tq, s1q, s1q)
            nssx = scratch.tile([16, N], F32, tag="nssx")
            nc.vector.scalar_tensor_tensor(out=nssx, in0=tq, scalar=invn_ap,
                                           in1=s2q, op0=OP.mult, op1=OP.subtract)
            # Ca = C*c^2*ssx  (additive constant for h = Ca - (c*G)^2)
            caf = scratch.tile([16, N], F32, tag="caf")
            nc.vector.tensor_scalar(out=caf, in0=nssx,
                                    scalar1=-(C * CSCALE * CSCALE),
                                    scalar2=None, op0=OP.mult)
            # LN = ln(c^2*ssx)  (log correction, per (r, i))
            lnf = scratch.tile([16, N], F32, tag="lnf")
            nc.scalar.activation(out=lnf, in_=nssx, func=AF.Ln,
                                 bias=0.0, scale=-(CSCALE * CSCALE))

            # transpose Ca and LN to [128, 8*16]: col ic*16 + (2h + m - 1)
            caT = statp.tile([128, 128], F32)
            lnT = statp.tile([128, 128], F32)
            with tc.tile_pool(name="tps", bufs=1, space="PSUM") as tps:
                pst = tps.tile([128, 128], F32, tag="pstA")
                for ic in range(8):
                    nc.tensor.transpose(pst[:, ic * 16:(ic + 1) * 16],
                                        in_=caf[:, ic * 128:(ic + 1) * 128],
                                        identity=id_s[0:16, 0:16])
                nc.scalar.copy(caT, pst)
                pst2 = tps.tile([128, 128], F32, tag="pstB")
                for ic in range(8):
                    nc.tensor.transpose(pst2[:, ic * 16:(ic + 1) * 16],
                                        in_=lnf[:, ic * 128:(ic + 1) * 128],
                                        identity=id_s[0:16, 0:16])
                nc.scalar.copy(lnT, pst2)
            # D[i, ic*8+h] = 0.5*(LN[2h] + LN[2h+1])  (free-dim stride-2 APs)
            dT = statp.tile([128, 64], F32)
            nc.vector.tensor_tensor(out=dT, in0=lnT[:, 0:128:2],
                                    in1=lnT[:, 1:128:2], op=OP.add)
            nc.vector.tensor_scalar(out=dT, in0=dT, scalar1=0.5,
                                    scalar2=None, op0=OP.mult)

            # m1 augmentation: overwrite q_projT row rb+63 (unused d=63) with
            # mx1.  Issued from the (idle) PE queue so these mx-gated DMAs
            # don't head-of-line-block the early yraw copies on sync.
            for h in range(H):
                ch, rb = h // 2, (h % 2) * 64
                nc.scalar.dma_start(out=qproj[ch][rb + 63:rb + 64, :],
                                    in_=mx[2 * h:2 * h + 1, :])

            # ---- Stage E: per-head Grams + elementwise --------------------
            prev = None
            with tc.tile_pool(name="head", bufs=2) as headp, \
                 tc.tile_pool(name="nsq", bufs=2) as nsq, \
                 tc.tile_pool(name="gps", bufs=1, space="PSUM") as gps:
                for h in range(H):
                    ch, rb = h // 2, (h % 2) * 64
                    yo1, yo2 = rb, 64 - rb
                    r1, r2 = 2 * h, 2 * h + 1

                    # Y raw: m1 block rows yo1..yo1+63 (k d=1..63 + aug),
                    #        m2 block rows yo2..yo2+62 (k d=2..63 + aug)
                    yraw = headp.tile([128, N], F16, tag="yraw")
                    nc.sync.dma_start(out=yraw[yo1:yo1 + 63, :],
                                        in_=kproj[ch][rb + 1:rb + 64, :])
                    nc.sync.dma_start(out=yraw[yo1 + 63:yo1 + 64, :],
                                        in_=ns1y[r1:r1 + 1, :])
                    nc.sync.dma_start(out=yraw[yo2:yo2 + 62, :],
                                        in_=kproj[ch][rb + 2:rb + 64, :])
                    nc.sync.dma_start(out=yraw[yo2 + 62:yo2 + 63, :],
                                        in_=ns1y[r2:r2 + 1, :])
                    hole = yo2 + 63  # the single uncovered row
                    nc.sync.dma_start(out=yraw[hole:hole + 1, :],
                                        in_=ns1y[r1:r1 + 1, :])

                    bb = headp.tile([128, N], F16, tag="bb")
                    nc.gpsimd.dma_start(
                        out=bb[yo1:yo1 + 64, :],
                        in_=betad[r1:r1 + 1, :].to_broadcast((64, N)))
                    nc.gpsimd.dma_start(
                        out=bb[yo2:yo2 + 64, :],
                        in_=betad[r2:r2 + 1, :].to_broadcast((64, N)))

                    yt = headp.tile([128, N], F16, tag="yt")
                    nc.vector.tensor_mul(yt, yraw, bb)

                    # X2: m2 lhsT block at rows yo2..yo2+62 (q d=0..61 + mx2)
                    x2 = headp.tile([128, N], F16, tag="x2")
                    nc.sync.dma_start(out=x2[yo2:yo2 + 62, :],
                                        in_=qproj[ch][rb:rb + 62, :])
                    nc.sync.dma_start(out=x2[yo2 + 62:yo2 + 63, :],
                                        in_=mx[r2:r2 + 1, :])

                    for pc in range(4):
                        ic0, ic1 = 2 * pc, 2 * pc + 1
                        psA = gps.tile([128, 2 * N], F32, tag="psA")
                        psB = gps.tile([128, 2 * N], F32, tag="psB")
                        for k, ic in ((0, ic0), (1, ic1)):
                            isl = slice(ic * 128, (ic + 1) * 128)
                            for jh in range(2):
                                jsl = slice(jh * 512, (jh + 1) * 512)
                                osl = slice(k * N + jh * 512,
                                            k * N + (jh + 1) * 512)
                                nc.tensor.matmul(psA[:, osl],
                                                 lhsT=qproj[ch][rb:rb + 64, isl],
                                                 rhs=yt[yo1:yo1 + 64, jsl],
                                                 start=True, stop=True)
                        for k, ic in ((0, ic0), (1, ic1)):
                            isl = slice(ic * 128, (ic + 1) * 128)
                            for jh in range(2):
                                jsl = slice(jh * 512, (jh + 1) * 512)
                                osl = slice(k * N + jh * 512,
                                            k * N + (jh + 1) * 512)
                                nc.tensor.matmul(psB[:, osl],
                                                 lhsT=x2[yo2:yo2 + 63, isl],
                                                 rhs=yt[yo2:yo2 + 63, jsl],
                                                 start=True, stop=True)

                        # sq_m = (c*G_m)^2, [128, 2048].  Every 3rd pair the
                        # G2 evacuation goes via DVE to offload the ACT
                        # engine (the bottleneck).
                        sg1 = nsq.tile([128, 2 * N], F16, tag="sg1")
                        nc.scalar.activation(out=sg1, in_=psA, func=AF.Square,
                                             bias=0.0, scale=CSCALE)
                        sg2 = nsq.tile([128, 2 * N], F16, tag="sg2")
                        if (h * 4 + pc) % 3 == 2:
                            e2 = nsq.tile([128, 2 * N], F16, tag="e2")
                            nc.vector.tensor_scalar(out=e2, in0=psB,
                                                    scalar1=CSCALE,
                                                    scalar2=None, op0=OP.mult)
                            nc.vector.tensor_tensor(out=sg2, in0=e2, in1=e2,
                                                    op=OP.mult)
                        else:
                            nc.scalar.activation(out=sg2, in_=psB,
                                                 func=AF.Square,
                                                 bias=0.0, scale=CSCALE)

                        # retire previous 2-pair group (one Ln over [128,4096])
                        # right after the new group's first sq emissions so
                        # the ACT queue never stalls on DVE
                        if prev is not None and pc % 2 == 0:
                            _retire(nc, nsq, out, dT, prev)
                            prev = None
                        if pc % 2 == 0:
                            ucur = nsq.tile([128, 4 * N], F16, tag="u4")
                            uinfo = []

                        # h_m = Ca_m - sq_m ; u = h1*h2
                        h1 = nsq.tile([128, 2 * N], F16, tag="h1")
                        h2 = nsq.tile([128, 2 * N], F16, tag="h2")
                        for k, ic in ((0, ic0), (1, ic1)):
                            ksl = slice(k * N, (k + 1) * N)
                            nc.vector.tensor_scalar(
                                out=h1[:, ksl], in0=sg1[:, ksl], scalar1=-1.0,
                                scalar2=caT[:, ic * 16 + r1:ic * 16 + r1 + 1],
                                op0=OP.mult, op1=OP.add)
                            nc.vector.tensor_scalar(
                                out=h2[:, ksl], in0=sg2[:, ksl], scalar1=-1.0,
                                scalar2=caT[:, ic * 16 + r2:ic * 16 + r2 + 1],
                                op0=OP.mult, op1=OP.add)
                        usl = slice((pc % 2) * 2 * N, (pc % 2 + 1) * 2 * N)
                        nc.vector.tensor_tensor(out=ucur[:, usl], in0=h1,
                                                in1=h2, op=OP.mult)
                        uinfo.extend([ic0, ic1])
                        if pc % 2 == 1:
                            prev = (ucur, h, uinfo)
                    if h == H - 1 and prev is not None:
                        _retire(nc, nsq, out, dT, prev)
                        prev = None
    nc.compile()
    return nc


def _retire(nc, nsq, out, dT, prev):
    """Ln + final scale/correction + store for a finished 2-pair group."""
    u4, h, ics = prev
    lt = nsq.tile([128, 4 * N], F16, tag="lt")
    nc.scalar.activation(out=lt, in_=u4, func=AF.Ln, bias=0.0, scale=1.0)
    # o = -0.5*ln(u) + D  (per-partition D differs per i-chunk)
    for k, ic in enumerate(ics):
        ksl = slice(k * N, (k + 1) * N)
        o = nsq.tile([128, N], F16, tag=f"o{k}")
        nc.vector.tensor_scalar(
            out=o, in0=lt[:, ksl], scalar1=-0.5,
            scalar2=dT[:, ic * 8 + h:ic * 8 + h + 1],
            op0=OP.mult, op1=OP.add)
        isl = slice(ic * 128, (ic + 1) * 128)
        nc.sync.dma_start(out=out[h, isl, :], in_=o)


_NC = None


def _get_nc():
    global _NC
    if _NC is None:
        _NC = _build_nc()
    return _NC


def _host_inputs(queries, keys, Wq_w, Wq_b, Wk_w, Wk_b):
    qT = np.ascontiguousarray(queries.transpose(0, 2, 1), dtype=np.float16)
    kT = np.ascontiguousarray(keys.transpose(0, 2, 1), dtype=np.float16)
    wqT = np.ascontiguousarray(Wq_w.T, dtype=np.float16)
    wkT = np.ascontiguousarray(Wk_w.T, dtype=np.float16)
    bq = np.ascontiguousarray(Wq_b.reshape(4, 128).T, dtype=np.float32)
    bk = np.ascontiguousarray(Wk_b.reshape(4, 128).T, dtype=np.float32)

    xmask = np.zeros((128, 64), dtype=np.float16)
    ymask = np.zeros((128, 64), dtype=np.float16)
    for c in range(4):
        for hp in range(2):
            for m in (1, 2):
                j = 4 * c + 2 * hp + (m - 1)      # output partition row r
                col = 16 * c + j                   # column within this chunk's mask
                rows = np.arange(hp * 64, hp * 64 + 64 - m)
                xmask[rows, col] = 1.0
                yrows = np.arange(hp * 64 + m, hp * 64 + 64)
                ymask[yrows, col] = 1.0

    invn = np.array([[1.0 / (64 - ((r % 2) + 1))] for r in range(16)],
                    dtype=np.float32)
    ident = np.eye(16, dtype=np.float32)

    shared = dict(wqT=wqT, wkT=wkT, bq=bq, bk=bk, xmask=xmask, ymask=ymask,
                  invn=invn, ident=ident)
    in_maps = []
    for b in range(B):
        m = dict(shared)
        m["qT"] = np.ascontiguousarray(qT[b])
        m["kT"] = np.ascontiguousarray(kT[b])
        in_maps.append(m)
    return in_maps


def kernel(queries, keys, Wq_w, Wq_b, Wk_w, Wk_b):
    nc = _get_nc()
    in_maps = _host_inputs(np.asarray(queries), np.asarray(keys),
                           np.asarray(Wq_w), np.asarray(Wq_b),
                           np.asarray(Wk_w), np.asarray(Wk_b))
    res = run_bass_kernel_spmd(nc, in_maps, core_ids=list(range(B)))
    out = np.stack([res.results[b]["out"].astype(np.float32) for b in range(B)],
                   axis=0)
    return out
